# revision 23
# baseline (speedup 1.0000x reference)
"""TRN2 Bass kernel for the ConceptualMambaBlock problem (pair-scan, v9).

Math (reference):
    x: [B=4, T=96, N=512, H=128] f32
    expanded = x @ W_exp.T + b_exp            # [B,T,N,2H]
    primary, gating = split(expanded, 2, -1)
    s_t = 0.9*s_{t-1} + 0.1*gating_t          # EMA along T
    out = (primary * sigmoid(s)) @ W_con.T + b_con

Strategy (v9, pair-compressed scan):
  - Shard (B x N/2) over 8 cores: core c -> batch c//2, node half c%2.
  - Host splits each core's x into even-t / odd-t planes [H, 2, 12288] bf16
    (node-major, t/2 fastest).  In shifted space sigma_t = 0.9 sigma_{t-1}
    + g_t with g = 0.1*W_g x and sigma_0 = g_0 - 0.9 b_g, the odd-t
    subsequence obeys sigma_{2j+1} = 0.81 sigma_{2j-1} + u_j with
    u_j = 0.9 g_{2j} + g_{2j+1}.  So:
      * PE accumulates u directly in PSUM (two matmuls with 0.09W / 0.1W
        weights + a tiny fixup matmul adding -0.81 b_g at node starts).
      * DVE tensor_tensor_scan runs over HALF the columns (decay 0.81),
        in node-aligned 480-col blocks => no inter-block carries at all.
      * sigma_even = (sigma_odd - g_odd)/0.9 is reconstructed on the PE:
        pg_neg = -g_odd (matmul) + I @ sigma_odd (accumulating matmul);
        the 1/0.9 rides the ACT sigmoid scale port.
  - Gate multiply y=(pp+b1p)*gate and the po+b2 output copy are split
    across DVE (STT / tensor_scalar from PSUM) and ACT+GpSimd (ppcopy+TT)
    by per-block tables MULT_ON_G / OC_ON_DVE.
  - PSUM: pg(shared u/neg tag) x3 + pp x3 + po x2 = 8 banks.
  - 26 blocks of 10 nodes (480 pair-cols; last block 6 nodes/288).
"""

import numpy as np
import ml_dtypes

import concourse.bacc as bacc
import concourse.bass as bass  # noqa: F401
import concourse.mybir as mybir
import concourse.tile as tile
from concourse.bass_utils import run_bass_kernel_spmd

F32 = mybir.dt.float32
BF16 = mybir.dt.bfloat16
AF = mybir.ActivationFunctionType
ALU = mybir.AluOpType

B, T, N, H = 4, 96, 512, 128
NCORES = 8
NLOC = N // 2            # 256 nodes per core
TH = T // 2              # 48 pair-steps per node
PAIRS = NLOC * TH        # 12288 pair-columns per core
NPB = 10                 # nodes per block
PB = NPB * TH            # 480 pair-cols per full block
NBLK = (NLOC + NPB - 1) // NPB   # 26 (last block 6 nodes = 288 cols)
GRP = 2                  # blocks per DMA group

# gate-multiply on GpSimd+ACT (True) vs DVE STT (False)
MULT_ON_G = [b % 13 not in (1, 4, 7, 10) and b < NBLK - 2
             for b in range(NBLK)]  # ~17 on GpSimd; tail on DVE
# output copy on DVE tensor_scalar (True) vs ACT (False)
OC_ON_DVE = list(MULT_ON_G)  # outcopy on DVE exactly when mult is on GpSimd

_NC_CACHE = None


def _blk(b):
    off = PB * b
    nn = min(NPB, NLOC - NPB * b)
    return off, nn * TH, nn


def _build():
    nc = bacc.Bacc()

    x_h = nc.dram_tensor("xt", [H, 2, PAIRS], BF16, kind="ExternalInput")
    wpack_h = nc.dram_tensor("wpack", [H, 6 * H], BF16, kind="ExternalInput")
    bnegrow_h = nc.dram_tensor("bnegrow", [1, H], BF16, kind="ExternalInput")
    bpack_h = nc.dram_tensor("bpack", [H, 3], F32, kind="ExternalInput")
    mask_h = nc.dram_tensor("mask", [H, PB], F32, kind="ExternalInput")
    out_h = nc.dram_tensor("out", [H, 2, PAIRS], BF16, kind="ExternalOutput")

    with tile.TileContext(nc) as tc:
        with (
            tc.tile_pool(name="consts", bufs=1) as cp,
            tc.tile_pool(name="io", bufs=2) as io,
            tc.tile_pool(name="midv", bufs=3) as midv,
            tc.tile_pool(name="mida", bufs=3) as mida,
            tc.tile_pool(name="midg", bufs=3) as midg,
            tc.tile_pool(name="ps", bufs=1, space="PSUM") as ps,
        ):
            wpack_sb = cp.tile([H, 6 * H], BF16, tag="wpack")
            nc.sync.dma_start(out=wpack_sb[:], in_=wpack_h[:, :])
            xg = {}

            def emit_load(b):
                # blocks 0-3: single-block loads (fast startup); then pairs
                if b >= NBLK or b in xg:
                    return
                if b < 4:
                    off, ncs, _ = _blk(b)
                    t = io.tile([H, 2, PB], BF16, tag="xts", name=f"xt{b}",
                                bufs=4)
                    nc.sync.dma_start(out=t[:, :, 0:ncs],
                                      in_=x_h[:, :, off : off + ncs])
                    xg[b] = (t, 0)
                    return
                b0 = 4 + ((b - 4) // GRP) * GRP
                o0 = PB * b0
                o1 = min(o0 + GRP * PB, PAIRS)
                t = io.tile([H, 2, GRP * PB], BF16, tag="xt", name=f"xt{b0}g",
                            bufs=4)
                nc.sync.dma_start(out=t[:, :, 0 : o1 - o0],
                                  in_=x_h[:, :, o0:o1])
                for bb in range(b0, min(b0 + GRP, NBLK)):
                    xg[bb] = (t, PB * bb - o0)

            bpack_sb = cp.tile([H, 3], F32, tag="bpack")
            nc.scalar.dma_start(out=bpack_sb[:], in_=bpack_h[:, :])
            mask_sb = cp.tile([H, PB], F32, tag="mask")
            nc.scalar.dma_start(out=mask_sb[:], in_=mask_h[:, :])
            bnegrow_sb = cp.tile([1, H], BF16, tag="bnegrow")
            nc.scalar.dma_start(out=bnegrow_sb[:], in_=bnegrow_h[:, :])
            emit_load(0)
            emit_load(1)

            ones_sb = cp.tile([1, PB], BF16, tag="ones")
            nc.gpsimd.memset(ones_sb[:], 1.0)

            # warm the ACT function tables before the pipeline needs them
            warm = cp.tile([H, 1], BF16, tag="warm")
            nc.scalar.activation(warm[:], wpack_sb[:, 0:1], AF.Sigmoid,
                                 bias=0.0, scale=1.0)
            nc.scalar.activation(warm[:], wpack_sb[:, 0:1], AF.Identity,
                                 bias=0.0, scale=1.0)

            w_ue = wpack_sb[:, 0:H]                # (0.09*Wg).T
            w_uo = wpack_sb[:, H : 2 * H]          # (0.1*Wg).T
            w_ng = wpack_sb[:, 2 * H : 3 * H]      # (-0.1*Wg).T
            w1p = wpack_sb[:, 3 * H : 4 * H]       # W_exp[:H].T
            w2 = wpack_sb[:, 4 * H : 5 * H]        # W_con.T
            ident = wpack_sb[:, 5 * H : 6 * H]     # I
            bg = bpack_sb[:, 0:1]
            b1p = bpack_sb[:, 1:2]
            b2 = bpack_sb[:, 2:3]

            st = {}
            ob = {}

            def xcol(b, plane):
                _, ncs, _ = _blk(b)
                t, lo = xg[b]
                return t[:, plane, lo : lo + ncs]

            def emit_u(b):
                _, ncs, nn = _blk(b)
                pgu = ps.tile([H, PB], F32, tag="pg", name=f"pgu{b}", bufs=4)
                nc.tensor.matmul(pgu[:, :ncs], lhsT=w_ue, rhs=xcol(b, 0),
                                 start=True, stop=False)
                nc.tensor.matmul(pgu[:, :ncs], lhsT=w_uo, rhs=xcol(b, 1),
                                 start=False, stop=False)
                nc.tensor.matmul(pgu[:, 0 : ncs : TH], lhsT=bnegrow_sb[:],
                                 rhs=ones_sb[:, 0:nn], start=False, stop=True)
                st[b] = {"pgu": pgu}

            def emit_scan(b):
                _, ncs, _ = _blk(b)
                pgu = st[b].pop("pgu")
                so = midv.tile([H, PB], BF16, tag="so", name=f"so{b}")
                nc.vector.tensor_tensor_scan(
                    out=so[:, :ncs], data0=mask_sb[:, :ncs], data1=pgu[:, :ncs],
                    initial=0.0, op0=ALU.mult, op1=ALU.add,
                )
                st[b]["so"] = so

            def emit_neg(b):
                _, ncs, _ = _blk(b)
                so = st[b].pop("so")
                pgn = ps.tile([H, PB], F32, tag="pg", name=f"pgn{b}", bufs=4)
                nc.tensor.matmul(pgn[:, :ncs], lhsT=w_ng, rhs=xcol(b, 1),
                                 start=True, stop=False)
                nc.tensor.matmul(pgn[:, :ncs], lhsT=ident, rhs=so[:, :ncs],
                                 start=False, stop=True)
                st[b]["pgn"] = pgn

            def emit_sig_o(b):
                _, ncs, _ = _blk(b)
                so = st[b]["so"]
                gtag = "gateg" if MULT_ON_G[b] else "gated"
                gate = mida.tile([H, 2, PB], BF16, tag=gtag, name=f"gate{b}")
                nc.scalar.activation(gate[:, 1, :ncs], so[:, :ncs], AF.Sigmoid,
                                     bias=bg, scale=1.0)
                st[b]["gate"] = gate

            def emit_sig_e(b):
                _, ncs, _ = _blk(b)
                pgn = st[b].pop("pgn")
                gate = st[b]["gate"]
                nc.scalar.activation(gate[:, 0, :ncs], pgn[:, :ncs], AF.Sigmoid,
                                     bias=bg, scale=1.0 / 0.9)

            def emit_mm1p(b):
                _, ncs, _ = _blk(b)
                pp = ps.tile([H, 2, 512], F32, tag="pp", name=f"pp{b}", bufs=1)
                nc.tensor.matmul(pp[:, 0, :ncs], lhsT=w1p, rhs=xcol(b, 0),
                                 start=True, stop=True)
                nc.tensor.matmul(pp[:, 1, :ncs], lhsT=w1p, rhs=xcol(b, 1),
                                 start=True, stop=True)
                st[b]["pp"] = pp

            def emit_ppcopy(b):
                _, ncs, _ = _blk(b)
                pp = st[b].pop("pp")
                pps = mida.tile([H, 2, PB], BF16, tag="pps", name=f"pps{b}")
                nc.scalar.activation(pps[:, :, :ncs], pp[:, :, :ncs],
                                     AF.Identity, bias=b1p, scale=1.0)
                st[b]["pps"] = pps

            def emit_mult(b):
                _, ncs, _ = _blk(b)
                gate = st[b].pop("gate")
                pool = midg if MULT_ON_G[b] else midv
                y = pool.tile([H, 2, PB], BF16, tag="y", name=f"y{b}")
                if MULT_ON_G[b]:
                    pps = st[b].pop("pps")
                    nc.gpsimd.tensor_tensor(out=y[:, :, :ncs],
                                            in0=pps[:, :, :ncs],
                                            in1=gate[:, :, :ncs], op=ALU.mult)
                else:
                    pp = st[b].pop("pp")
                    nc.vector.scalar_tensor_tensor(
                        out=y[:, :, :ncs], in0=pp[:, :, :ncs], scalar=b1p,
                        in1=gate[:, :, :ncs], op0=ALU.add, op1=ALU.mult,
                    )
                st[b]["y"] = y

            def emit_mm2(b):
                _, ncs, _ = _blk(b)
                y = st[b]["y"]
                po = ps.tile([H, 2, 512], F32, tag="po", name=f"po{b}", bufs=1)
                nc.tensor.matmul(po[:, 0, :ncs], lhsT=w2, rhs=y[:, 0, :ncs],
                                 start=True, stop=True)
                nc.tensor.matmul(po[:, 1, :ncs], lhsT=w2, rhs=y[:, 1, :ncs],
                                 start=True, stop=True)
                st[b]["po"] = po

            def emit_out(b):
                _, ncs, _ = _blk(b)
                po = st[b]["po"]
                gi = b // GRP
                if b % GRP == 0:
                    ob[gi] = io.tile([H, 2, GRP * PB], BF16, tag="ob",
                                     name=f"ob{gi}", bufs=3)
                lo = (b % GRP) * PB
                dst = ob[gi][:, :, lo : lo + ncs]
                if OC_ON_DVE[b]:
                    nc.vector.tensor_scalar(
                        out=dst, in0=po[:, :, :ncs],
                        scalar1=b2, scalar2=None, op0=ALU.add,
                    )
                else:
                    nc.scalar.activation(dst, po[:, :, :ncs],
                                         AF.Identity, bias=b2, scale=1.0)
                del st[b]

            def emit_outdma(gi):
                o0 = gi * GRP * PB
                o1 = min(o0 + GRP * PB, PAIRS)
                nc.gpsimd.dma_start(out=out_h[:, :, o0:o1],
                                    in_=ob.pop(gi)[:, :, 0 : o1 - o0])

            NGRP = (NBLK + GRP - 1) // GRP
            emit_load(2)
            emit_load(3)

            STEPS = NBLK + 1
            for s in range(STEPS):
                if s % GRP == 0:
                    emit_load(s + 4)
                    emit_load(s + 4 + GRP)

                b_out = s - 4      # mm2 + outcopy
                b_mult = s - 2
                b_mid = s - 1      # sig + neg/I + mm1p + ppcopy
                b_new = s
                gi_dma = (s - 6) // 2 if (s - 6) % 2 == 0 else -1

                if 0 <= b_out < NBLK:
                    emit_mm2(b_out)
                if 0 <= b_mid < NBLK:
                    emit_mm1p(b_mid)
                    emit_sig_o(b_mid)
                    if MULT_ON_G[b_mid]:
                        emit_ppcopy(b_mid)
                if 0 <= b_mult < NBLK:
                    emit_mult(b_mult)
                if 0 <= b_out < NBLK:
                    emit_out(b_out)
                if 0 <= gi_dma < NGRP:
                    emit_outdma(gi_dma)
                if b_new < NBLK:
                    emit_u(b_new)
                if 0 <= b_mid < NBLK:
                    emit_neg(b_mid)
                    emit_sig_e(b_mid)
                if b_new < NBLK:
                    emit_scan(b_new)

            # collapsed drain: run the remaining stages back-to-back
            emit_mult(NBLK - 1)
            for b in (NBLK - 3, NBLK - 2, NBLK - 1):
                emit_mm2(b)
                emit_out(b)
                gi = b // GRP
                if gi in ob and (b % GRP == GRP - 1 or b == NBLK - 1):
                    o0 = gi * GRP * PB
                    o1 = min(o0 + GRP * PB, PAIRS)
                    nc.sync.dma_start(out=out_h[:, :, o0:o1],
                                      in_=ob.pop(gi)[:, :, 0 : o1 - o0])
            for gi in range(NGRP):
                if gi in ob:
                    emit_outdma(gi)

    nc.finalize()
    return nc


def _get_nc():
    global _NC_CACHE
    if _NC_CACHE is None:
        _NC_CACHE = _build()
    return _NC_CACHE


def _in_maps(x, W_exp, b_exp, W_con, b_con):
    Wg = 0.1 * W_exp[H:, :]
    wpack = np.concatenate(
        [
            (0.9 * Wg).T, Wg.T, (-Wg).T,
            W_exp[:H, :].T, W_con.T, np.eye(H, dtype=np.float32),
        ],
        axis=1,
    ).astype(ml_dtypes.bfloat16)
    wpack = np.ascontiguousarray(wpack)
    bg = b_exp[H:]
    bpack = np.stack([bg, b_exp[:H], b_con], axis=1).astype(np.float32)
    bpack = np.ascontiguousarray(bpack)
    bnegrow = np.ascontiguousarray(
        (-0.81 * bg)[None, :].astype(ml_dtypes.bfloat16)
    )

    mask = np.full((H, PB), 0.81, np.float32)
    mask[:, 0::TH] = 0.0
    mask = np.ascontiguousarray(mask)

    maps = []
    for c in range(NCORES):
        bb, nh = c // 2, c % 2
        xs = x[bb, :, nh * NLOC : (nh + 1) * NLOC, :]  # [T, NLOC, H]
        xT = xs.transpose(2, 1, 0)                     # [H, NLOC, T]
        xpk = np.stack(
            [xT[:, :, 0::2].reshape(H, PAIRS), xT[:, :, 1::2].reshape(H, PAIRS)],
            axis=1,
        )
        maps.append(
            {
                "xt": np.ascontiguousarray(xpk.astype(ml_dtypes.bfloat16)),
                "wpack": wpack,
                "bpack": bpack,
                "mask": mask,
                "bnegrow": bnegrow,
            }
        )
    return maps


def run_spmd(x, W_exp, b_exp, W_con, b_con, **spmd_kwargs):
    """Run the 8-core kernel; returns (full_output, BassKernelResults)."""
    maps = _in_maps(x, W_exp, b_exp, W_con, b_con)
    res = run_bass_kernel_spmd(
        _get_nc(), maps, core_ids=list(range(NCORES)), **spmd_kwargs
    )
    out = np.empty((B, T, N, H), dtype=np.float32)
    for c in range(NCORES):
        bb, nh = c // 2, c % 2
        oT = np.asarray(res.results[c]["out"]).astype(np.float32)
        full = np.empty((H, NLOC, T), dtype=np.float32)
        full[:, :, 0::2] = oT[:, 0, :].reshape(H, NLOC, TH)
        full[:, :, 1::2] = oT[:, 1, :].reshape(H, NLOC, TH)
        out[bb, :, nh * NLOC : (nh + 1) * NLOC, :] = full.transpose(2, 1, 0)
    return out, res


def kernel(spatial_temporal_representation, W_exp, b_exp, W_con, b_con):
    out, _ = run_spmd(
        np.asarray(spatial_temporal_representation, dtype=np.float32),
        np.asarray(W_exp, dtype=np.float32),
        np.asarray(b_exp, dtype=np.float32),
        np.asarray(W_con, dtype=np.float32),
        np.asarray(b_con, dtype=np.float32),
    )
    return out
